# revision 43
# baseline (speedup 1.0000x reference)
"""Differential multi-head attention on 8 Trainium2 NeuronCores.

Sharding: tensor-parallel over heads x data-parallel over batch.
Core c handles batch b = c//4 and real heads [4*(c%4), 4*(c%4)+4).
Each core computes a partial output (its 256 attention features through
the output projection); the host sums the 4 partials per batch.

Per-core dataflow (all matmuls bf16 with fp32 PSUM accumulation):
  qT/kT = W @ x.T          [feat, s] layout (feat on partitions)
  v     = x @ Wv.T         [s, feat] layout, plus a ones column per head
  ST_c  = k_c^T q_c        scores transposed: [keys, q] (keys on partitions)
  PT_c  = exp(ST_c)        on ScalarE; scores bounded ~6.5 so exp never
                           overflows and no max-subtraction is needed
  O_c   = PT_c^T @ v_aug   PV with P STATIONARY (full 128-wide loads) and
                           v_aug [keys,65] moving: out [q, 65] accumulates
                           over the 16 key tiles directly in PSUM; col 64
                           is the softmax denominator (ones-column trick).
                           This halves the billed PE time vs v-stationary
                           (65-col moving vs 256-col) and the output needs
                           no transpose before normalization.
  per-q: O = O1/r1 - lam*O2/r2; rms = exp(-0.5*ln(ssq/64 + eps));
  attn = O*rms (subln_w, 1-lam_init and the q scaling are folded into the
  weights on the host)
  out += attnT @ Wo'       partial over this core's 256 features

ScalarE is the bottleneck (~260us of exp busy): scores are built in
[128,1536] PSUM slabs (4/6/6 key tiles x 256 queries; unit 0 uses
4/4/4/4 so its later exps are not gated by the x DMA) so each exp
instruction covers 1024-1536 columns, amortizing the fixed per-
instruction SBUF-access overhead.

All 192 slabs from the 32 (head, query-chunk) units form ONE global
pipeline. Per iteration: exp(g) on ScalarE (the pacing engine), fill
(g+1) on PE into the other score-psum buffer, deferred DVE/ScalarE/PE
work due at g, at most one extra QKV projection round, then PV
retirement. PV retires ~19 slabs behind the exp stream (so unit 0's
PV -- which needs all 16 v tiles = 13.7us of projection rounds -- does
not starve ScalarE during warm-up) and ramps down to lag 3 near the
end to keep the tail short. pt tiles are 6-deep to cover the PV lag.

PSUM (8 banks): score slabs 2x[128,1536] (6) + PV-out 1x[128,512]
(freed early by one DVE copy at the head of normalize) + projection
rounds 1x[128,512].

DMA: the modeled DMA device serializes ALL transfers, and each
dma_start costs ~625ns on the shared HWDGE sequencer, so weights load
as single consolidated transfers and x as four column-slices (first
slice -> kn0/qn0/first fills start ~7us in); queue choice orders the
serial device by criticality. A burst of dummy matmuls at t~0 ramps
the PE p-state before the first projection rounds. Output partials are
written bf16 (host accumulates in f32; adds ~2e-4 to a 6e-3 rel err).

Modeled per-core time (TRN2 TimelineSim): ~297us; ScalarE 87% busy.
"""

import math
import sys

sys.path.insert(0, "/opt/trn_rl_repo")

from contextlib import ExitStack

import ml_dtypes
import numpy as np

import concourse.bacc as bacc
import concourse.mybir as mybir
import concourse.tile as tile
from concourse.bass_utils import run_bass_kernel_spmd

# The kernel's only transcendentals are Exp and Ln; make the activation
# table-set chooser prefer the one set containing both, so a single
# ACT_TABLE_LOAD covers the whole kernel (the default order picks
# exp_and_others for Exp, forcing table reloads around the rms Ln).
_orig_get_activation_tables = bacc.get_activation_tables


def _tables_ln_exp_pinned(arch):
    # Keep dict ORDER identical (act_func_set_id is a positional index into
    # act_info.json), but remove Exp/Ln from every other set so the chooser
    # can only satisfy them from the combined set.
    t = dict(_orig_get_activation_tables(arch))
    pref = "natural_log_exp_and_others"
    if pref not in t:
        return t
    A = mybir.ActivationFunctionType
    out = {}
    for k, v in t.items():
        if k != pref:
            v = {f for f in v if f not in (A.Exp, A.Ln)}
        out[k] = v
    return out


bacc.get_activation_tables = _tables_ln_exp_pinned

F32 = mybir.dt.float32
BF16 = mybir.dt.bfloat16
ALU = mybir.AluOpType
ACT = mybir.ActivationFunctionType

E = 1024          # embed dim
S = 2048          # sequence length
B = 2             # batch
H = 16            # real heads
D = 32            # head dim (per component)
NCORES = 8
HPC = 4           # real heads per core
FPC = HPC * 2 * D  # features per core for q/k/v slices = 256
LAMBDA_INIT = 0.8 - 0.6 * math.exp(-0.3 * 12)
EPS = 1e-5

QC = 256          # query-chunk width
NQC = S // QC     # 8
NST = QC // 128   # q-subtiles per chunk
NKT = S // 128    # 16 key tiles
# key-tile slabs per component: exp instruction = one slab (cols = kt*QC)
SLABS = [(0, 4), (4, 10), (10, 16)]
NSLAB = len(SLABS)
OTW = 66          # ot column stride (65 cols + 1 pad for 8B alignment)


def build_kernel(reps: int = 1):
    nc = bacc.Bacc("TRN2", target_bir_lowering=False, debug=False,
                   num_devices=NCORES)
    xT = nc.dram_tensor("xT", [E, S], BF16, kind="ExternalInput")
    wq = nc.dram_tensor("wq", [E, FPC], BF16, kind="ExternalInput")
    wk = nc.dram_tensor("wk", [E, FPC], BF16, kind="ExternalInput")
    wv = nc.dram_tensor("wv", [E, FPC], BF16, kind="ExternalInput")
    wo = nc.dram_tensor("wo", [FPC, E], BF16, kind="ExternalInput")
    lam = nc.dram_tensor("lam", [128, 2], F32, kind="ExternalInput")
    idb = nc.dram_tensor("idb", [128, 128], BF16, kind="ExternalInput")
    out = nc.dram_tensor("out", [S, E], BF16, kind="ExternalOutput")

    with tile.TileContext(nc) as tc, ExitStack() as ctx:
        cpool = ctx.enter_context(tc.tile_pool(name="consts", bufs=1))
        ipool = ctx.enter_context(tc.tile_pool(name="inputs", bufs=1))
        qpool = ctx.enter_context(tc.tile_pool(name="qkv", bufs=1))
        ptp = ctx.enter_context(tc.tile_pool(name="pt", bufs=6))
        apool = ctx.enter_context(tc.tile_pool(name="araw", bufs=1))
        wpool = ctx.enter_context(tc.tile_pool(name="work", bufs=3))
        # PSUM (8 banks of 2KB): score slabs 2x[128,1536] (6 banks),
        # PV-out 1x[128,512] (freed fast by a Pool copy in normalize),
        # projection rounds 1x[128,512].
        ps_st = ctx.enter_context(tc.tile_pool(name="pst", bufs=2, space="PSUM"))
        ps_ot = ctx.enter_context(tc.tile_pool(name="pot", bufs=1, space="PSUM"))
        ps_rd = ctx.enter_context(tc.tile_pool(name="prd", bufs=1, space="PSUM"))

        # Consolidated DMAs: every dma_start costs ~625ns on the single
        # serialized HWDGE descriptor engine, so the 8 row-blocks of each
        # weight live in ONE [128, 8*256] tile loaded by ONE transfer
        # (dram rows (kb p) -> tile cols (kb c)). x is split into 4
        # transfers on 4 different queues so its 32KB/partition overlaps.
        # Order matters: wk and x gate the first score fills; lam/idb/wo
        # are only needed mid-kernel.
        wk_all = ipool.tile([128, 8 * FPC], BF16, tag="wk")
        nc.sync.dma_start(
            wk_all.rearrange("p (kb c) -> p kb c", kb=8),
            wk.ap().rearrange("(kb p) c -> p kb c", kb=8))
        wq_all = ipool.tile([128, 8 * FPC], BF16, tag="wq")
        nc.scalar.dma_start(
            wq_all.rearrange("p (kb c) -> p kb c", kb=8),
            wq.ap().rearrange("(kb p) c -> p kb c", kb=8))
        # x arrives in COLUMN slices (all 8 contraction blocks of 512
        # seq-columns each): kt/qt/v columns depend only on the matching
        # x columns, so kn0/qn0 and the first score fills start after the
        # first slice (~3us of the serial DMA device) instead of after
        # the full 11.6us x transfer.
        # tile layout is SLICE-major — cols = nch*4096 + kb*512 + c — so
        # each slice's write range is contiguous (range-granular dep
        # tracking would otherwise serialize readers on all 4 slices).
        x_all = ipool.tile([128, 8 * S], BF16, tag="x")
        xT_r = xT.ap().rearrange("(kb p) s -> p kb s", kb=8)
        # slice 0 rides the gpsimd queue (fastest issue, ~0.7us) so it
        # wins the serial DMA device right after wk; later slices follow
        # in criticality order.
        xq = (nc.gpsimd, nc.sync, nc.scalar, nc.gpsimd)
        for i in range(4):
            dst = x_all[:, 4096 * i:4096 * (i + 1)]
            xq[i].dma_start(dst.rearrange("p (kb c) -> p kb c", kb=8),
                            xT_r[:, :, 512 * i:512 * (i + 1)])

        def x_cols(kb, c0, width):
            sl, off = divmod(c0, 512)
            assert off + width <= 512
            base = sl * 4096 + kb * 512 + off
            return x_all[:, base:base + width]
        wv_all = ipool.tile([128, 8 * FPC], BF16, tag="wv")
        nc.scalar.dma_start(
            wv_all.rearrange("p (kb c) -> p kb c", kb=8),
            wv.ap().rearrange("(kb p) c -> p kb c", kb=8))
        lamt = cpool.tile([128, 2], F32, tag="lam")
        nc.sync.dma_start(lamt[:], lam.ap())
        lam_sb = lamt[:, 0:1]
        eps_sb = lamt[:, 1:2]
        idb_sb = cpool.tile([128, 128], BF16, tag="idb")
        nc.sync.dma_start(idb_sb[:], idb.ap())
        wo_sb = []
        for fb in range(2):
            t = ipool.tile([128, E], BF16, tag=f"wo{fb}", name="t")
            nc.sync.dma_start(t[:], wo.ap()[fb * 128:(fb + 1) * 128, :])
            wo_sb.append(t)

        def wqk_slice(w_all, kb, fb):
            return w_all[:, kb * FPC + fb * 128:kb * FPC + (fb + 1) * 128]

        # PE p-state warmup: ~6us of dummy matmuls on a memset tile so the
        # first real projection rounds run at full clock (the cost model
        # ramps PE over ~3us of continuous activity).
        wmt = cpool.tile([128, 128], BF16, tag="wmt")
        nc.gpsimd.memset(wmt[:], 0.0)
        wm_ps = ps_rd.tile([128, 512], F32, tag="rd", name="wm_ps")
        for _ in range(40):
            nc.tensor.matmul(wm_ps[:, 0:128], wmt[:], wmt[:],
                             start=True, stop=True)

        for _rep in range(reps):
            # ---------------- QKV projection rounds ----------------
            qt, kt = [None, None], [None, None]
            vt = [None] * NKT

            def proj_qk_round(dname, dst_list, w_all, fb, nch):
                if dst_list[fb] is None:
                    dst_list[fb] = qpool.tile([128, S], BF16,
                                              tag=f"{dname}{fb}", name="t")
                t = dst_list[fb]
                ps = ps_rd.tile([128, 512], F32, tag="rd")
                for kb in range(8):
                    nc.tensor.matmul(
                        ps[:], wqk_slice(w_all, kb, fb),
                        x_cols(kb, nch * 512, 512),
                        start=(kb == 0), stop=(kb == 7))
                nc.vector.tensor_copy(
                    t[:, nch * 512:(nch + 1) * 512], ps[:])

            def proj_v(st):
                t = qpool.tile([128, HPC * 65], BF16, tag=f"v{st}")
                vt[st] = t
                ps = ps_rd.tile([128, 512], F32, tag="rd")
                for kb in range(8):
                    nc.tensor.matmul(
                        ps[:, 0:FPC],
                        x_cols(kb, st * 128, 128),
                        wv_all[:, kb * FPC:(kb + 1) * FPC],
                        start=(kb == 0), stop=(kb == 7))
                tv = t.rearrange("p (h x) -> p h x", x=65)
                nc.vector.tensor_copy(
                    tv[:, :, 0:64],
                    ps[:, 0:FPC].rearrange("p (h x) -> p h x", x=64))
                nc.vector.memset(tv[:, :, 64:65], 1.0)

            # ---------------- attention helpers ----------------
            def fill_slab(u, c, si, parity):
                """Score matmuls for one (component, slab): [keys, q]."""
                k0, k1 = u["slabs"][si]
                fb = u["fb"]
                for nch in range((k0 * 128) // 512, (k1 * 128 - 1) // 512 + 1):
                    ensure_qk("kt", kt, wk_all, fb, nch)
                ensure_qk("qt", qt, wq_all, fb, (u["qc"] * QC) // 512)
                stp = ps_st.tile([128, 1536], F32, tag="st")
                off = u["off1"] if c == 0 else u["off2"]
                tp = (off, 0) if off == 96 else None
                for j in range(k1 - k0):
                    ktile = k0 + j
                    nc.tensor.matmul(
                        stp[:, j * QC:(j + 1) * QC],
                        kt[u["fb"]][off:off + 32,
                                    ktile * 128:(ktile + 1) * 128],
                        qt[u["fb"]][off:off + 32,
                                    u["qc"] * QC:(u["qc"] + 1) * QC],
                        start=True, stop=True, tile_position=tp)
                return stp

            def exp_slab(u, c, si, stp):
                k0, k1 = u["slabs"][si]
                pt = u["pt1"] if c == 0 else u["pt2"]
                nc.scalar.activation(
                    pt[:, k0 * QC:k1 * QC], stp[:, 0:(k1 - k0) * QC],
                    ACT.Exp)

            def ensure_v(k0, k1):
                for st in range(k0, k1):
                    if vt[st] is None:
                        proj_v(st)

            def pv_slab(u, c, si):
                """PV for one (component, slab): P stationary, v moving.

                out[q,65] accumulates over key tiles in ONE psum bank; the
                very first matmul of the unit uses start=True (clears the
                bank's has_written bits), every other region's first write
                relies on still-clear bits (start=False overwrites).
                """
                k0, k1 = u["slabs"][si]
                ensure_v(k0, k1)
                pt = u["pt1"] if c == 0 else u["pt2"]
                h = u["h"]
                for st in range(NST):
                    col = OTW * (2 * st + c)
                    for j in range(k0, k1):
                        nc.tensor.matmul(
                            u["ot"][:, col:col + 65],
                            pt[:, j * QC + st * 128:j * QC + st * 128 + 128],
                            vt[j][:, h * 65:(h + 1) * 65],
                            start=(j == 0 and st == 0 and c == 0),
                            stop=(j == NKT - 1),
                            skip_group_check=True)

            def make_normalize(u):
                ot, h = u["ot"], u["h"]
                araw, ssq = u["araw"], u["ssq"]

                def _normalize():
                    # one Pool copy releases the single ot psum bank fast;
                    # the DVE math reads the SBUF copy
                    otc = wpool.tile([128, OTW * 4], F32, tag="otc")
                    nc.vector.tensor_copy(otc[:], ot[:, 0:OTW * 4])
                    for st in range(NST):
                        c1o = OTW * (2 * st)
                        c2o = OTW * (2 * st + 1)
                        inv1 = wpool.tile([128, 1], F32, tag="inv1")
                        inv2 = wpool.tile([128, 1], F32, tag="inv2")
                        nc.vector.reciprocal(inv1[:], otc[:, c1o + 64:c1o + 65])
                        nc.vector.reciprocal(inv2[:], otc[:, c2o + 64:c2o + 65])
                        o1n = wpool.tile([128, 64], F32, tag="o1n")
                        o2n = wpool.tile([128, 64], F32, tag="o2n")
                        nc.vector.tensor_scalar_mul(
                            o1n[:], otc[:, c1o:c1o + 64], inv1[:])
                        nc.vector.tensor_scalar(
                            o2n[:], otc[:, c2o:c2o + 64],
                            inv2[:], lam_sb, op0=ALU.mult, op1=ALU.mult)
                        nc.vector.tensor_sub(
                            araw[:, st, h, :], o1n[:], o2n[:])
                        sqs = wpool.tile([128, 64], F32, tag="sqs")
                        nc.vector.tensor_mul(
                            sqs[:], araw[:, st, h, :], araw[:, st, h, :])
                        nc.vector.tensor_reduce(
                            ssq[:, st * HPC + h:st * HPC + h + 1], sqs[:],
                            axis=mybir.AxisListType.X, op=ALU.add)
                return _normalize

            def make_rms(qc, araw, ssq, box):
                def _rms():
                    # rms scale = exp(-0.5 * ln(ssq/64 + eps))
                    rln = wpool.tile([128, NST * HPC], F32, tag="rln")
                    rmsi = wpool.tile([128, NST * HPC], F32, tag="rmsi")
                    nc.scalar.activation(rln[:], ssq[:], ACT.Ln,
                                         scale=1.0 / 64.0, bias=eps_sb)
                    nc.scalar.activation(rmsi[:], rln[:], ACT.Exp, scale=-0.5)
                    attn_bf = wpool.tile([128, NST, HPC, 64], BF16, tag="abf")
                    for st in range(NST):
                        for h in range(HPC):
                            nc.vector.tensor_scalar_mul(
                                attn_bf[:, st, h, :], araw[:, st, h, :],
                                rmsi[:, st * HPC + h:st * HPC + h + 1])
                    box.append(attn_bf)
                return _rms

            def make_proj(qc, st, box):
                def _proj():
                    attn_bf = box[0]
                    att_flat = attn_bf.rearrange("p s h d -> p s (h d)")
                    atps = ps_rd.tile([128, 256], BF16, tag="rd", name="atps")
                    nc.tensor.transpose(atps[:, 0:128],
                                        att_flat[:, st, 0:128], idb_sb[:])
                    nc.tensor.transpose(atps[:, 128:256],
                                        att_flat[:, st, 128:256], idb_sb[:])
                    at0 = wpool.tile([128, 128], BF16, tag="at0")
                    at1 = wpool.tile([128, 128], BF16, tag="at1")
                    nc.vector.tensor_copy(at0[:], atps[:, 0:128])
                    nc.vector.tensor_copy(at1[:], atps[:, 128:256])
                    row = (qc * NST + st) * 128
                    for ec in range(2):
                        ops = ps_rd.tile([128, 512], F32, tag="rd")
                        nc.tensor.matmul(
                            ops[:], at0[:],
                            wo_sb[0][:, ec * 512:(ec + 1) * 512],
                            start=True, stop=False)
                        nc.tensor.matmul(
                            ops[:], at1[:],
                            wo_sb[1][:, ec * 512:(ec + 1) * 512],
                            start=False, stop=True)
                        osb = wpool.tile([128, 512], BF16, tag="osb")
                        nc.vector.tensor_copy(osb[:], ops[:])
                        nc.sync.dma_start(
                            out.ap()[row:row + 128,
                                     ec * 512:(ec + 1) * 512], osb[:])
                return _proj

            # ---------------- global slab pipeline ----------------
            # Slabs from all units form one stream. Per iteration g:
            # exp(g) [ActE pacing], fill(g+1) [score-psum ping-pong one
            # slab ahead], deferred work due at g, at most one extra
            # projection round that has come due, then PV retirement.
            # PV retires with a LARGE lag behind the exp stream early on
            # (unit 0''s PV needs all 16 v tiles = 13.7us of PE rounds, so
            # the deadline must sit ~32 slabs behind to give PE room) and
            # ramps down to lag 3 near the end so the tail stays short.
            from collections import defaultdict, deque

            # Heads 0-1 (fb0) over all chunks first, then per-chunk heads
            # 2-3: fb1 q/k projections spread over the heads-0/1 runway.
            units = [(qc, h) for h in (0, 1) for qc in range(NQC)]
            units += [(qc, h) for qc in range(NQC) for h in (2, 3)]
            # unit 0 runs 4-ktile slabs: its later exps would otherwise
            # stall on the kn2 projection round behind the x DMA
            useq = {0: [(c, si) for si in range(4) for c in range(2)]}
            dseq = [(c, si) for si in range(NSLAB) for c in range(2)]
            SLABS4 = [(0, 4), (4, 8), (8, 12), (12, 16)]
            uslabs = {0: SLABS4}

            def unit_seq(ui):
                return useq.get(ui, dseq)

            def unit_slabs(ui):
                return uslabs.get(ui, SLABS)

            glist = [(ui, c, si) for ui in range(len(units))
                     for (c, si) in unit_seq(ui)]
            NG = len(glist)
            PVLAG = 19

            def retire_target(g):
                # lag PVLAG early, ramping (slope 2) to lag 3 well before
                # the last exps so the 2nd-to-last chunk's normalize/rms/
                # proj chain overlaps the exp stream instead of trailing it
                return min(g - 3, max(g - PVLAG, 2 * g - (NG - 18)))

            qc_state = {}
            ustate = {}

            def get_unit(ui):
                if ui in ustate:
                    return ustate[ui]
                qc, h = units[ui]
                if qc not in qc_state:
                    qc_state[qc] = (
                        apool.tile([128, NST, HPC, 64], F32,
                                   tag=f"araw{qc}", name="araw"),
                        apool.tile([128, NST * HPC], F32,
                                   tag=f"ssq{qc}", name="ssq"))
                araw_t, ssq_t = qc_state[qc]
                u = {"qc": qc, "h": h, "fb": h // 2,
                     "off1": 64 * (h % 2), "off2": 64 * (h % 2) + 32,
                     "araw": araw_t, "ssq": ssq_t, "slabs": unit_slabs(ui),
                     "nspu": len(unit_seq(ui)),
                     "pt1": None, "pt2": None, "ot": None, "npv": 0}
                ustate[ui] = u
                return u

            # (due_iteration, item) — popped when g >= due.
            items = [(0, ("qk", "kt", kt, wk_all, 0, 1)),
                     (1, ("qk", "kt", kt, wk_all, 0, 2)),
                     (3, ("qk", "kt", kt, wk_all, 0, 3)),
                     (9, ("qk", "qt", qt, wq_all, 0, 1)),
                     (20, ("qk", "qt", qt, wq_all, 0, 2)),
                     (33, ("qk", "qt", qt, wq_all, 0, 3)),
                     (92, ("qk", "qt", qt, wq_all, 1, 0)),
                     (117, ("qk", "qt", qt, wq_all, 1, 1)),
                     (141, ("qk", "qt", qt, wq_all, 1, 2)),
                     (165, ("qk", "qt", qt, wq_all, 1, 3))]
            items += [(8 + j, ("v", st)) for j, st in enumerate(range(16))]
            items += [(80 + 3 * j, ("qk", "kt", kt, wk_all, 1, j))
                      for j in range(4)]
            extra = deque(sorted(items, key=lambda it: it[0]))

            qk_done = set()

            def ensure_qk(dname, dst, w_all, fb, nch):
                if (dname, fb, nch) in qk_done:
                    return
                qk_done.add((dname, fb, nch))
                proj_qk_round(dname, dst, w_all, fb, nch)

            def run_extra(g):
                if not extra or extra[0][0] > g:
                    return
                item = extra.popleft()[1]
                if item[0] == "v":
                    if vt[item[1]] is None:
                        proj_v(item[1])
                else:
                    _, dname, dst, w_all, fb, nch = item
                    ensure_qk(dname, dst, w_all, fb, nch)

            pending = defaultdict(list)

            def do_fill(g):
                ui, c, si = glist[g]
                return fill_slab(get_unit(ui), c, si, g % 2)

            def do_pv(p, g):
                ui, c, si = glist[p]
                u = get_unit(ui)
                if u["ot"] is None:
                    u["ot"] = ps_ot.tile([128, 512], F32,
                                         tag="ot", name="ot")
                pv_slab(u, c, si)
                u["npv"] += 1
                if u["npv"] == u["nspu"]:
                    pending[g + 1].append(make_normalize(u))
                    if u["h"] == HPC - 1:
                        qc = u["qc"]
                        araw_t, ssq_t = qc_state[qc]
                        box = []
                        last = ui == len(units) - 1
                        o1, o2, o3 = (1, 2, 3) if last else (4, 6, 8)
                        pending[g + o1].append(
                            make_rms(qc, araw_t, ssq_t, box))
                        pending[g + o2].append(make_proj(qc, 0, box))
                        pending[g + o3].append(make_proj(qc, 1, box))

            # Prologue: kn0 and qn0 with their kb-matmuls interleaved so
            # each accumulation step fires as soon as its x block lands.
            qk_done.add(("kt", 0, 0))
            qk_done.add(("qt", 0, 0))
            kt[0] = qpool.tile([128, S], BF16, tag="kt0", name="t")
            qt[0] = qpool.tile([128, S], BF16, tag="qt0", name="t")
            psk = ps_rd.tile([128, 512], F32, tag="rd", name="psk")
            for kb in range(8):
                nc.tensor.matmul(
                    psk[:], wqk_slice(wk_all, kb, 0),
                    x_cols(kb, 0, 512),
                    start=(kb == 0), stop=(kb == 7))
            nc.vector.tensor_copy(kt[0][:, 0:512], psk[:])
            psq = ps_rd.tile([128, 512], F32, tag="rd", name="psq")
            for kb in range(8):
                nc.tensor.matmul(
                    psq[:], wqk_slice(wq_all, kb, 0),
                    x_cols(kb, 0, 512),
                    start=(kb == 0), stop=(kb == 7))
            nc.vector.tensor_copy(qt[0][:, 0:256], psq[:, 0:256])
            nc.vector.tensor_copy(qt[0][:, 256:512], psq[:, 256:512])
            st_tiles = {0: do_fill(0)}

            retired = 0
            g = 0
            KEEPALIVE = NG - 14
            while retired < NG or pending:
                if g < NG:
                    ui, c, si = glist[g]
                    u = get_unit(ui)
                    if u["pt1"] is None:
                        u["pt1"] = ptp.tile([128, NKT * QC], BF16,
                                            tag="pt1", name="pt1")
                        u["pt2"] = ptp.tile([128, NKT * QC], BF16,
                                            tag="pt2", name="pt2")
                    exp_slab(u, c, si, st_tiles.pop(g))
                if g + 1 < NG:
                    st_tiles[g + 1] = do_fill(g + 1)
                for fn in pending.pop(g, []):
                    fn()
                run_extra(g)

                tgt = min(retire_target(g), NG)
                while retired < tgt:
                    do_pv(retired, g)
                    retired += 1
                g += 1
            for gk in sorted(pending):
                for fn in pending.pop(gk, []):
                    fn()
            qc_state.clear()
            ustate.clear()
    nc.compile()
    return nc


def _prep_core_inputs(inputs, core):
    x = np.asarray(inputs["x"], np.float32)
    Wq = np.asarray(inputs["Wq"], np.float32)
    Wk = np.asarray(inputs["Wk"], np.float32)
    Wv = np.asarray(inputs["Wv"], np.float32)
    Wo = np.asarray(inputs["Wo"], np.float32)
    subln_w = np.asarray(inputs["subln_w"], np.float32)
    b, hg = core // 4, core % 4
    sl = slice(FPC * hg, FPC * (hg + 1))
    bf = ml_dtypes.bfloat16
    scaling = D ** -0.5
    lam_full = float(
        np.exp(np.sum(np.asarray(inputs["lambda_q1"], np.float64)
                      * np.asarray(inputs["lambda_k1"], np.float64)))
        - np.exp(np.sum(np.asarray(inputs["lambda_q2"], np.float64)
                        * np.asarray(inputs["lambda_k2"], np.float64)))
        + LAMBDA_INIT)
    wo_scale = (np.tile(subln_w, HPC)[:, None] * (1.0 - LAMBDA_INIT))
    return {
        "xT": np.ascontiguousarray(x[b].T).astype(bf),
        "wq": np.ascontiguousarray(Wq[sl].T * scaling).astype(bf),
        "wk": np.ascontiguousarray(Wk[sl].T).astype(bf),
        "wv": np.ascontiguousarray(Wv[sl].T).astype(bf),
        "wo": np.ascontiguousarray(Wo[:, sl].T * wo_scale).astype(bf),
        "lam": np.stack([np.full(128, lam_full, np.float32),
                         np.full(128, EPS, np.float32)], axis=1),
        "idb": np.eye(128, dtype=ml_dtypes.bfloat16),
    }


_CACHED = {}


def _get_kernel(reps=1):
    if reps not in _CACHED:
        _CACHED[reps] = build_kernel(reps)
    return _CACHED[reps]


def run_on_cores(inputs, reps=1):
    nc = _get_kernel(reps)
    in_maps = [_prep_core_inputs(inputs, c) for c in range(NCORES)]
    res = run_bass_kernel_spmd(nc, in_maps, core_ids=list(range(NCORES)))
    return res


def kernel(**inputs) -> np.ndarray:
    res = run_on_cores(inputs)
    out = np.zeros((B, S, E), np.float32)
    for c in range(NCORES):
        out[c // 4] += np.asarray(res.results[c]["out"], np.float32)
    return out
